# revision 17
# baseline (speedup 1.0000x reference)
"""Trainium2 Bass kernel for nn_MultiHeadAttention_89008902243131.

Module: 1x1-conv QKV (+BN+SiLU) -> 8-head attention over N=4096 tokens
(head_dim 16) -> 1x1-conv proj (+BN+SiLU).  B=2, C=128, H=W=64.

Sharding: tensor-parallel over the 8 heads (one head per core, both
batches).  Each core computes its head's full attention; the only
cross-core exchange is an AllToAll of the per-head attention outputs
(channel shards -> query-column shards) before the proj conv, so every
core applies the full proj to its own 512-column slice of the output.

Device-side layout tricks:
 - BN is folded into conv weights/bias on the host.
 - q/k are produced by the conv directly in a "replicated x2" layout
   (head rows at partition offsets 0 and 32) so the NxN score matmuls
   can run as two concurrent row-tiled (K=16) PE matmuls.
 - S^T is computed (keys on partitions, queries free) so softmax's exp
   runs on ScalarE straight out of PSUM, and P^T feeds the P@V matmul
   with no transposes anywhere.  Softmax denominators come free from a
   ones-column appended to V; exp needs no max-subtraction (logits are
   small by construction, |scale*S| << 80).
"""

import numpy as np
from contextlib import ExitStack

import concourse.bass as bass
import concourse.tile as tile
from concourse import bacc, mybir
from concourse.bass_utils import run_bass_kernel_spmd

F32 = mybir.dt.float32
AF = mybir.ActivationFunctionType

B = 2
C = 128
N = 4096          # H*W tokens
NH = 8            # heads
HD = 16           # head dim
NCORES = 8
QB = 512          # query block (fp32 moving-operand max)
NQB = N // QB     # 8 query blocks
KC = 128          # key chunk (PE M dim)
NKC = N // KC     # 32 key chunks
SCALE = float(HD) ** -0.5
BN_EPS = 1e-3


def build_program():
    # Bacc (not plain Bass): its compile() runs the wait-legalization /
    # event-semaphore / act-table passes walrus codegen requires.
    nc = bacc.Bacc(
        "TRN2",
        target_bir_lowering=False,
        debug=False,
        num_devices=NCORES,
    )

    x_io = nc.dram_tensor("x", [B, C, N], F32, kind="ExternalInput").ap()
    wq_io = nc.dram_tensor("wq", [C, 64], F32, kind="ExternalInput").ap()
    wk_io = nc.dram_tensor("wk", [C, 64], F32, kind="ExternalInput").ap()
    wv_io = nc.dram_tensor("wv", [C, HD], F32, kind="ExternalInput").ap()
    bq_io = nc.dram_tensor("bq", [64, 1], F32, kind="ExternalInput").ap()
    bk_io = nc.dram_tensor("bk", [64, 1], F32, kind="ExternalInput").ap()
    bv_io = nc.dram_tensor("bv", [HD, 1], F32, kind="ExternalInput").ap()
    wp_io = nc.dram_tensor("wp", [C, C], F32, kind="ExternalInput").ap()
    bp_io = nc.dram_tensor("bp", [C, 1], F32, kind="ExternalInput").ap()
    id_io = nc.dram_tensor("ident", [HD, HD], F32, kind="ExternalInput").ap()
    ones_io = nc.dram_tensor("ones16", [1, HD], F32, kind="ExternalInput").ap()
    y_io = nc.dram_tensor("y", [B, C, QB], F32, kind="ExternalOutput").ap()

    with tile.TileContext(nc) as tc, ExitStack() as ctx:
        const = ctx.enter_context(tc.tile_pool(name="const", bufs=1))
        big = ctx.enter_context(tc.tile_pool(name="big", bufs=1))
        pre_ps = ctx.enter_context(tc.tile_pool(name="pre_ps", bufs=2, space="PSUM"))
        s_ps_pool = ctx.enter_context(tc.tile_pool(name="s_ps", bufs=2, space="PSUM"))
        o_ps_pool = ctx.enter_context(tc.tile_pool(name="o_ps", bufs=2, space="PSUM"))
        p_pool = ctx.enter_context(tc.tile_pool(name="p_pool", bufs=2))
        sm = ctx.enter_context(tc.tile_pool(name="sm", bufs=2))
        dram = ctx.enter_context(tc.tile_pool(name="dram", bufs=1, space="DRAM"))

        # ---- constants -------------------------------------------------
        wq_sb = const.tile([C, 64], F32)
        nc.sync.dma_start(wq_sb[:], wq_io[:])
        wk_sb = const.tile([C, 64], F32)
        nc.sync.dma_start(wk_sb[:], wk_io[:])
        wv_sb = const.tile([C, HD], F32)
        nc.sync.dma_start(wv_sb[:], wv_io[:])
        bq_sb = const.tile([64, 1], F32)
        nc.sync.dma_start(bq_sb[:], bq_io[:])
        bk_sb = const.tile([64, 1], F32)
        nc.sync.dma_start(bk_sb[:], bk_io[:])
        bv_sb = const.tile([HD, 1], F32)
        nc.sync.dma_start(bv_sb[:], bv_io[:])
        wp_sb = const.tile([C, C], F32)
        nc.sync.dma_start(wp_sb[:], wp_io[:])
        bp_sb = const.tile([C, 1], F32)
        nc.sync.dma_start(bp_sb[:], bp_io[:])
        id_sb = const.tile([HD, HD], F32)
        nc.sync.dma_start(id_sb[:], id_io[:])
        ones_sb = const.tile([1, HD], F32)
        nc.sync.dma_start(ones_sb[:], ones_io[:])

        # ---- static SBUF tensors --------------------------------------
        x_sb = big.tile([C, B * N], F32)
        q_sb = big.tile([64, B * N], F32)   # head q replicated at part 0 and 32
        k_sb = big.tile([64, B * N], F32)   # head k replicated at part 0 and 32
        v_sb = big.tile([HD, B * N], F32)   # head v, [hd, tokens]
        # V^T chunks, 33 wide: cols 0..15 = V^T, col 32 = ones (so the
        # softmax-denominator row lands on the 32-aligned PSUM partition
        # the DVE is allowed to read); cols 16..31 produce junk partitions
        # 16..31 of o_ps that are never read.
        v_t = big.tile([C, B * NKC * 33], F32)
        a_sb = big.tile([HD, B * N], F32)   # divided O^T (attention out rows)

        # ones columns for the softmax-denominator trick: fill everything
        # with 1.0 first; the V^T copies below overwrite cols 0..15 of
        # each 17-wide chunk, leaving col 16 == 1.0.
        nc.gpsimd.memset(v_t[:], 1.0)

        # ---- load x ----------------------------------------------------
        for b in range(B):
            for blk in range(NQB):
                nc.sync.dma_start(
                    x_sb[:, b * N + blk * QB : b * N + (blk + 1) * QB],
                    x_io[b][:, blk * QB : (blk + 1) * QB],
                )

        # ---- qkv conv + BN + SiLU -------------------------------------
        for b in range(B):
            for blk in range(NQB):
                cols = slice(b * N + blk * QB, b * N + (blk + 1) * QB)
                xb = x_sb[:, cols]
                ps_q = pre_ps.tile([64, QB], F32, tag="pre")
                nc.tensor.matmul(ps_q[:], wq_sb[:], xb)
                nc.scalar.activation(q_sb[:, cols], ps_q[:], AF.Silu, bias=bq_sb[:])
                ps_k = pre_ps.tile([64, QB], F32, tag="pre")
                nc.tensor.matmul(ps_k[:], wk_sb[:], xb)
                nc.scalar.activation(k_sb[:, cols], ps_k[:], AF.Silu, bias=bk_sb[:])
                ps_v = pre_ps.tile([HD, QB], F32, tag="pre")
                nc.tensor.matmul(ps_v[:], wv_sb[:], xb)
                nc.scalar.activation(v_sb[:, cols], ps_v[:], AF.Silu, bias=bv_sb[:])

        # ---- V^T chunks (PE transpose via identity) -------------------
        for b in range(B):
            for c in range(NKC):
                tp = pre_ps.tile([C, HD], F32, tag="pre")
                nc.tensor.transpose(
                    tp[:], v_sb[:, b * N + c * KC : b * N + (c + 1) * KC], id_sb[:]
                )
                off = (b * NKC + c) * 33
                nc.vector.tensor_copy(v_t[:, off : off + HD], tp[:])

        # ---- attention + output ---------------------------------------
        cc_outs = []
        for b in range(B):
            for qb in range(NQB):
                qcols = slice(b * N + qb * QB, b * N + (qb + 1) * QB)
                o_ps = o_ps_pool.tile([64, QB], F32)
                for g in range(NKC // 2):
                    c0, c1 = 2 * g, 2 * g + 1
                    s_ps = s_ps_pool.tile([C, 2 * QB], F32)
                    nc.tensor.matmul(
                        s_ps[:, 0:QB],
                        k_sb[0:HD, b * N + c0 * KC : b * N + (c0 + 1) * KC],
                        q_sb[0:HD, qcols],
                    )
                    nc.tensor.matmul(
                        s_ps[:, QB : 2 * QB],
                        k_sb[32 : 32 + HD, b * N + c1 * KC : b * N + (c1 + 1) * KC],
                        q_sb[32 : 32 + HD, qcols],
                    )
                    p_sb = p_pool.tile([C, 2 * QB], F32)
                    nc.scalar.activation(p_sb[:], s_ps[:], AF.Exp, scale=SCALE)
                    for j, cc in enumerate((c0, c1)):
                        voff = (b * NKC + cc) * 33
                        nc.tensor.matmul(
                            o_ps[0:33, :],
                            v_t[:, voff : voff + 33],
                            p_sb[:, j * QB : (j + 1) * QB],
                            start=(g == 0 and j == 0),
                            stop=(g == NKC // 2 - 1 and j == 1),
                            skip_group_check=True,
                        )
                recip = sm.tile([1, QB], F32)
                nc.vector.reciprocal(recip[:], o_ps[32:33, :])
                # broadcast 1/sum across the 16 head-dim partitions via a
                # K=1 PE matmul: ones16.T @ recip -> [16, QB] in PSUM
                bc_ps = pre_ps.tile([HD, QB], F32, tag="pre")
                nc.tensor.matmul(bc_ps[:], ones_sb[:], recip[:])
                bcast = sm.tile([HD, QB], F32)
                nc.vector.tensor_copy(bcast[:], bc_ps[:])
                nc.vector.tensor_mul(a_sb[0:HD, qcols], o_ps[0:HD, :], bcast[:])

            # ---- AllToAll: head-channel shards -> query-column shards --
            cc_in = dram.tile([NCORES, HD, QB], F32, name=f"cc_in{b}")
            for j in range(NCORES):
                nc.sync.dma_start(
                    cc_in[j], a_sb[0:HD, b * N + j * QB : b * N + (j + 1) * QB]
                )
            cc_out = dram.tile([NCORES, HD, QB], F32, name=f"cc_out{b}")
            nc.gpsimd.collective_compute(
                "AllToAll",
                mybir.AluOpType.bypass,
                replica_groups=[list(range(NCORES))],
                ins=[cc_in.opt()],
                outs=[cc_out.opt()],
            )
            cc_outs.append(cc_out)

        # ---- proj conv + BN + SiLU on this core's column slice --------
        for b in range(B):
            asl = sm.tile([C, QB], F32, tag="asl")
            nc.sync.dma_start(asl[:], cc_outs[b].rearrange("a b c -> (a b) c"))
            ps_y = pre_ps.tile([C, QB], F32, tag="pre")
            nc.tensor.matmul(ps_y[:], wp_sb[:], asl[:])
            y_sb = sm.tile([C, QB], F32, tag="ysb")
            nc.scalar.activation(y_sb[:], ps_y[:], AF.Silu, bias=bp_sb[:])
            nc.sync.dma_start(y_io[b], y_sb[:])

    nc.compile()
    return nc


def _host_prep(inputs):
    x = np.ascontiguousarray(np.asarray(inputs["x"], np.float32)).reshape(B, C, N)
    w_qkv = np.asarray(inputs["w_qkv"], np.float32)
    sc = np.asarray(inputs["qkv_gamma"], np.float32) / np.sqrt(
        np.asarray(inputs["qkv_var"], np.float32) + BN_EPS
    )
    w_qkv_eff = w_qkv * sc[:, None]
    b_qkv = (
        np.asarray(inputs["qkv_beta"], np.float32)
        - np.asarray(inputs["qkv_mean"], np.float32) * sc
    )
    scp = np.asarray(inputs["proj_gamma"], np.float32) / np.sqrt(
        np.asarray(inputs["proj_var"], np.float32) + BN_EPS
    )
    w_proj_eff = np.asarray(inputs["w_proj"], np.float32) * scp[:, None]
    b_proj = (
        np.asarray(inputs["proj_beta"], np.float32)
        - np.asarray(inputs["proj_mean"], np.float32) * scp
    )

    wp = np.ascontiguousarray(w_proj_eff.T)  # lhsT [c, o]
    bp = np.ascontiguousarray(b_proj[:, None])
    ident = np.eye(HD, dtype=np.float32)

    in_maps = []
    for h in range(NCORES):
        wq_h = w_qkv_eff[h * HD : (h + 1) * HD, :]          # [16, 128]
        wk_h = w_qkv_eff[C + h * HD : C + (h + 1) * HD, :]
        wv_h = w_qkv_eff[2 * C + h * HD : 2 * C + (h + 1) * HD, :]
        bq_h = b_qkv[h * HD : (h + 1) * HD]
        bk_h = b_qkv[C + h * HD : C + (h + 1) * HD]
        bv_h = b_qkv[2 * C + h * HD : 2 * C + (h + 1) * HD]

        wq = np.zeros((C, 64), np.float32)
        wk = np.zeros((C, 64), np.float32)
        bq = np.zeros((64, 1), np.float32)
        bk = np.zeros((64, 1), np.float32)
        for r in range(2):
            wq[:, 32 * r : 32 * r + HD] = wq_h.T
            wk[:, 32 * r : 32 * r + HD] = wk_h.T
            bq[32 * r : 32 * r + HD, 0] = bq_h
            bk[32 * r : 32 * r + HD, 0] = bk_h

        in_maps.append(
            {
                "x": x,
                "wq": wq,
                "wk": wk,
                "wv": np.ascontiguousarray(wv_h.T),
                "bq": bq,
                "bk": bk,
                "bv": np.ascontiguousarray(bv_h[:, None]),
                "wp": wp,
                "bp": bp,
                "ident": ident,
                "ones16": np.ones((1, HD), np.float32),
            }
        )
    return in_maps


def run(inputs, trace=False):
    """Build + run on the 8 cores; returns (y_full, BassKernelResults)."""
    in_maps = _host_prep(inputs)
    nc = build_program()
    res = run_bass_kernel_spmd(
        nc, in_maps, core_ids=list(range(NCORES)), trace=trace
    )
    y = np.zeros((B, C, N), np.float32)
    for h in range(NCORES):
        y[:, :, h * QB : (h + 1) * QB] = res.results[h]["y"]
    return y.reshape(B, C, 64, 64), res


def kernel(**inputs):
    y, _ = run(inputs)
    return y
